# revision 28
# baseline (speedup 1.0000x reference)
"""Trainium2 Bass kernel for nn_DeepGCN (GENConv softmax-aggr, 4 layers).

Sharding: edges partitioned by destination-node range across 8 cores (each
core owns 1250 consecutive nodes and all edges pointing into them); per layer
the full gather table (node features) is rebuilt with an AllGather collective.

Per 125-node block, softmax aggregation is computed as PE matmuls with
per-tile one-hot dst matrices accumulating [sum(exp), sum(msg*exp)] in PSUM;
h[src] rows are fetched with dma_gather from the DRAM table and edge_attr is
added inline by an accumulating DMA (CCE add).

Self-contained: only needs numpy + the installed concourse/bass stack.
"""

import os
import numpy as np

# ---- problem constants (hardcoded per the task spec) ----
N = 10000
E = 320000
H = 128
L = 4
G = 64            # num graphs
C = 10            # num classes
MSG_EPS = 1e-7
LN_EPS = 1e-5

NCORES = 8
NPC = N // NCORES          # 1250 nodes per core
NB = 10                    # node blocks per core
BN = NPC // NB             # 125 nodes per block
P = BN                     # partition count for node-side ops

_cache = {}
last_results = None        # BassKernelResults of the most recent run (for test.py)


def _build_program(TPB, t_vals):
    import concourse.bacc as bacc
    import concourse.tile as tile
    import concourse.mybir as mybir

    f32 = mybir.dt.float32
    i16 = mybir.dt.int16
    ALU = mybir.AluOpType
    AF = mybir.ActivationFunctionType
    AX = mybir.AxisListType

    NT = NB * TPB              # edge tiles per core
    E_PAD = NT * 128

    nc = bacc.Bacc("TRN2", target_bir_lowering=False, debug=False,
                   num_devices=NCORES,
                   num_swdge_queues=4 if os.environ.get('GNN_QUEUES','4')=='4' else 1)

    # ---------------- dram I/O ----------------
    bf16 = mybir.dt.bfloat16
    ea = nc.dram_tensor("ea", [128, NT, H], bf16, kind="ExternalInput")
    idxs_d = nc.dram_tensor("idxs", [128, E_PAD // 16], i16, kind="ExternalInput")
    dstrel_d = nc.dram_tensor("dstrel", [128, NT], f32, kind="ExternalInput")
    xT_d = nc.dram_tensor("xT", [128, NPC], f32, kind="ExternalInput")
    goh_d = nc.dram_tensor("goh", [128, NB * G], f32, kind="ExternalInput")
    nvals_d = nc.dram_tensor("nvals", [128, 128], bf16, kind="ExternalInput")
    eye_d = nc.dram_tensor("eye", [128, 128], f32, kind="ExternalInput")
    encW_d = nc.dram_tensor("encW", [128, H], f32, kind="ExternalInput")
    encb_d = nc.dram_tensor("encb", [1, H], f32, kind="ExternalInput")
    w1_d = nc.dram_tensor("w1", [128, L, 2 * H], f32, kind="ExternalInput")
    b1_d = nc.dram_tensor("b1", [1, L, 2 * H], f32, kind="ExternalInput")
    w2a_d = nc.dram_tensor("w2a", [128, L, H], f32, kind="ExternalInput")
    w2b_d = nc.dram_tensor("w2b", [128, L, H], f32, kind="ExternalInput")
    b2_d = nc.dram_tensor("b2", [1, L, H], f32, kind="ExternalInput")
    mlpg_d = nc.dram_tensor("mlpg", [128, L, 2 * H], f32, kind="ExternalInput")
    mlpb_d = nc.dram_tensor("mlpb", [128, L, 2 * H], f32, kind="ExternalInput")
    lng_d = nc.dram_tensor("lng", [128, L, H], f32, kind="ExternalInput")
    lnb_d = nc.dram_tensor("lnb", [128, L, H], f32, kind="ExternalInput")
    linW_d = nc.dram_tensor("linW", [128, C], f32, kind="ExternalInput")
    linb_d = nc.dram_tensor("linb", [1, C], f32, kind="ExternalInput")

    out_logits = nc.dram_tensor("out_logits", [G, C], f32, kind="ExternalOutput")
    out_pooled = nc.dram_tensor("out_pooled", [G, H], f32, kind="ExternalOutput")

    tables = [(nc.dram_tensor(f"table{l}a", [N // 2, H], bf16, kind="Internal"),
               nc.dram_tensor(f"table{l}b", [N // 2, H], bf16, kind="Internal"))
              for l in range(L)]
    pool_red = nc.dram_tensor("pool_red", [128, G], f32, kind="Internal")

    # ---------------- sbuf persistents ----------------
    def sbt(name, shape, dtype=f32):
        return nc.alloc_sbuf_tensor(name, list(shape), dtype)

    idx_sb = sbt("idx_sb", [128, E_PAD // 16], i16)
    dstrel_sb = sbt("dstrel_sb", [128, NT])
    nvals_sb = sbt("nvals_sb", [128, 128], bf16)
    eye_sb = sbt("eye_sb", [128, 128])
    ones_sb = sbt("ones_sb", [1, 128])
    onesb_sb = sbt("onesb_sb", [1, 128], bf16)
    goh_sb = sbt("goh_sb", [128, NB * G])
    xT_sb = sbt("xT_sb", [128, NPC])
    encW_sb = sbt("encW_sb", [128, H])
    encb_sb = sbt("encb_sb", [1, H])
    w1_sb = sbt("w1_sb", [128, L, 2 * H])
    b1_sb = sbt("b1_sb", [1, L, 2 * H])
    w2a_sb = sbt("w2a_sb", [128, L, H])
    w2b_sb = sbt("w2b_sb", [128, L, H])
    b2_sb = sbt("b2_sb", [1, L, H])
    mlpg_sb = sbt("mlpg_sb", [128, L, 2 * H])
    mlpb_sb = sbt("mlpb_sb", [128, L, 2 * H])
    lng_sb = sbt("lng_sb", [128, L, H])
    lnb_sb = sbt("lnb_sb", [128, L, H])
    linW_sb = sbt("linW_sb", [128, C])
    linb_sb = sbt("linb_sb", [1, C])

    conv_own = sbt("conv_own", [128, NB, H])     # gather-table slab of own nodes
    h_a = sbt("h_a", [128, NB, H])
    h_b = sbt("h_b", [128, NB, H])
    zs = sbt("zs", [128, NB, 2 * H])             # mm1 output (sbuf copy)
    hpT = sbt("hpT", [128, NB, H])               # h' transposed
    uT0 = sbt("uT0", [128, NB, H])
    uT1 = sbt("uT1", [128, NB, H])
    sA = sbt("sA", [128, NB * 2 * H])            # scratch
    sB = sbt("sB", [128, NB * 2 * H])
    sC = sbt("sC", [128, NB * H])
    sD = sbt("sD", [128, NB * H])
    poolT_sb = sbt("poolT_sb", [128, G])
    st1 = sbt("st1", [128, NB])
    st2 = sbt("st2", [128, NB])
    st3 = sbt("st3", [128, NB])
    st4 = sbt("st4", [128, NB])

    _const_cols = {}

    def const_col(val, parts=128):
        """[parts, 1] SBUF column filled with `val` (for activation bias).

        Must be called inside the TileContext (emits a memset on first use).
        """
        val = float(val)
        if val not in _const_cols:
            t = sbt(f"constc_{len(_const_cols)}", [128, 1])
            nc.vector.memset(t[:], val)
            _const_cols[val] = t
        return _const_cols[val][0:parts, 0:1]

    with tile.TileContext(nc) as tc:
        with (
            tc.tile_pool(name="pg", bufs=int(os.environ.get("GNN_BUFS","2"))) as pg,
            tc.tile_pool(name="pq", bufs=int(os.environ.get("GNN_BUFS","2"))) as pq,
            tc.tile_pool(name="pm", bufs=(int(os.environ.get("GNN_BUFS","2")) if TPB <= 40 else 1)) as pm,
            tc.tile_pool(name="poh", bufs=4) as poh,
            tc.tile_pool(name="pacc", bufs=2, space="PSUM") as pacc,
            tc.tile_pool(name="ptp", bufs=2, space="PSUM") as ptp,
            tc.tile_pool(name="pz", bufs=2, space="PSUM") as pz,
            tc.tile_pool(name="po", bufs=2, space="PSUM") as po,
            tc.tile_pool(name="pdram", bufs=2, space="DRAM") as pdram,
        ):
            # ---------------- load persistents ----------------
            for dst_t, src_t in [
                (idx_sb, idxs_d), (dstrel_sb, dstrel_d), (nvals_sb, nvals_d),
                (eye_sb, eye_d), (goh_sb, goh_d), (xT_sb, xT_d),
                (encW_sb, encW_d), (encb_sb, encb_d),
                (w1_sb, w1_d), (b1_sb, b1_d), (w2a_sb, w2a_d),
                (w2b_sb, w2b_d), (b2_sb, b2_d), (mlpg_sb, mlpg_d),
                (mlpb_sb, mlpb_d), (lng_sb, lng_d), (lnb_sb, lnb_d),
                (linW_sb, linW_d), (linb_sb, linb_d),
            ]:
                nc.sync.dma_start(dst_t[:], src_t[:])
            nc.vector.memset(ones_sb[:], 1.0)
            nc.vector.memset(onesb_sb[:], 1.0)

            def ag_half(table_half, half):
                """AllGather blocks [5*half, 5*half+5) of conv_own."""
                slab = pdram.tile([NPC // 2, H], bf16)
                slab_v = slab[:].rearrange("(b i) h -> i b h", i=BN)
                nc.gpsimd.dma_start(
                    slab_v, conv_own[0:P, 5 * half:5 * half + 5, :])
                nc.gpsimd.collective_compute(
                    "AllGather", ALU.bypass,
                    replica_groups=[list(range(NCORES))],
                    ins=[slab.opt()], outs=[table_half[:]],
                )

            def allgather_slab(table):
                ag_half(table[0], 0)
                ag_half(table[1], 1)

            def emit_ln_relu(src_full, src_blk, F, g_ap, b_ap, dst_full):
                """dst = relu(LN(src) * g + b); src viewed as [P, NB, F]."""
                inv = 1.0 / F
                nf = NB * F
                sqv = sA[0:P, 0:nf]
                nc.vector.tensor_mul(sqv, src_full, src_full)
                nc.vector.reduce_sum(st1[0:P, :], src_full, axis=AX.X)
                nc.vector.reduce_sum(
                    st2[0:P, :],
                    sA[0:P, 0:nf].rearrange("p (b f) -> p b f", f=F),
                    axis=AX.X)
                nc.vector.tensor_scalar(st1[0:P, :], st1[0:P, :], -inv, None,
                                        ALU.mult)
                nc.vector.tensor_mul(st3[0:P, :], st1[0:P, :], st1[0:P, :])
                nc.vector.tensor_scalar(st2[0:P, :], st2[0:P, :], inv, None,
                                        ALU.mult)
                nc.vector.tensor_sub(st4[0:P, :], st2[0:P, :], st3[0:P, :])
                nc.scalar.activation(st2[0:P, :], st4[0:P, :], AF.Ln,
                                     bias=const_col(LN_EPS, P))
                nc.scalar.activation(st3[0:P, :], st2[0:P, :], AF.Exp,
                                     scale=-0.5)
                for b in range(NB):
                    nc.vector.tensor_scalar(
                        sB[0:P, b * F:(b + 1) * F], src_blk(b),
                        st1[0:P, b:b + 1], st3[0:P, b:b + 1],
                        ALU.add, ALU.mult)
                for b in range(NB):
                    nc.vector.tensor_mul(sA[0:P, b * F:(b + 1) * F],
                                         sB[0:P, b * F:(b + 1) * F], g_ap)
                for b in range(NB):
                    nc.vector.tensor_add(sB[0:P, b * F:(b + 1) * F],
                                         sA[0:P, b * F:(b + 1) * F], b_ap)
                nc.scalar.activation(dst_full, sB[0:P, 0:nf], AF.Relu)

            # ---------------- encoder: h0 = x @ encW + encb ----------------
            for b in range(NB):
                ps = po.tile([128, 128], f32, tag="o")
                nc.tensor.matmul(ps[0:P, 0:H], xT_sb[:, b * BN:(b + 1) * BN],
                                 encW_sb[:], start=True, stop=False)
                nc.tensor.matmul(ps[0:P, 0:H], ones_sb[0:1, 0:P],
                                 encb_sb[:], start=False, stop=True)
                nc.scalar.copy(conv_own[0:P, b, :], ps[0:P, 0:H])
            allgather_slab(tables[0])

            # ---------------- layers ----------------
            n_layers = int(os.environ.get("GNN_LAYERS", str(L)))
            h_bufs = [h_a, h_b]
            for l in range(n_layers):
                t_l = float(t_vals[l])
                abs_t = abs(t_l) if t_l != 0.0 else 1e-12
                sign_t = 1.0 if t_l >= 0 else -1.0
                table = tables[l]
                h_new = h_bufs[l % 2]
                h_prev = h_bufs[(l + 1) % 2]

                # ---- edge + per-block node phase (pipelined) ----
                GCH = 8
                slab_a = pdram.tile([NPC // 2, H], bf16, tag="slab_a")
                slab_b = pdram.tile([NPC // 2, H], bf16, tag="slab_b")
                slabs = [slab_a, slab_b]
                slab_vs = [slab_a[:].rearrange("(b i) h -> i b h", i=BN),
                           slab_b[:].rearrange("(b i) h -> i b h", i=BN)]

                def ln_block(src_ap, F, g_ap, b_ap, dst_ap, b, sq_ap, u_ap,
                             v_ap):
                    """dst = relu(LN(src)*g+b) for one 125-node block.

                    src/dst: [P, F] APs (SBUF). Uses st1..st4 column b.
                    """
                    inv = 1.0 / F
                    c1 = st1[0:P, b:b + 1]
                    c2 = st2[0:P, b:b + 1]
                    c3 = st3[0:P, b:b + 1]
                    c4 = st4[0:P, b:b + 1]
                    nc.vector.tensor_mul(sq_ap, src_ap, src_ap)
                    nc.vector.reduce_sum(c1, src_ap, axis=AX.X)
                    nc.vector.reduce_sum(c2, sq_ap, axis=AX.X)
                    nc.vector.tensor_scalar(c1, c1, -inv, None, ALU.mult)
                    nc.vector.tensor_mul(c3, c1, c1)
                    nc.vector.tensor_scalar(c2, c2, inv, None, ALU.mult)
                    nc.vector.tensor_sub(c4, c2, c3)
                    nc.scalar.activation(c2, c4, AF.Ln, bias=const_col(LN_EPS, P))
                    nc.scalar.activation(c3, c2, AF.Exp, scale=-0.5)
                    nc.vector.tensor_scalar(u_ap, src_ap, c1, c3,
                                            ALU.add, ALU.mult)
                    nc.vector.tensor_mul(v_ap, u_ap, g_ap)
                    nc.vector.tensor_add(u_ap, v_ap, b_ap)
                    nc.scalar.activation(dst_ap, u_ap, AF.Relu)

                HTPB = TPB // 2
                for b in range(NB):
                    acc = pacc.tile([128, 2 * H], f32, tag="acc")
                    g = pg.tile([128, TPB, H], bf16, tag="g")
                    qn = 0
                    for grp in range(2):
                        done = 0
                        while done < HTPB:
                            ck = min(GCH, HTPB - done)
                            t0 = b * TPB + grp * HTPB + done
                            nc.gpsimd.dma_gather(
                                g[:, grp * HTPB + done:
                                  grp * HTPB + done + ck, :],
                                table[grp][:],
                                idx_sb[:, t0 * 8:(t0 + ck) * 8],
                                ck * 128, ck * 128, H,
                                queue_num=(qn % 4) if os.environ.get('GNN_QUEUES','4')=='4' else 0)
                            qn += 1
                            done += ck
                    # CCE accumulate caps at 2048 elements/partition per
                    # transfer -> split into <=16-tile pieces.
                    a0 = 0
                    while a0 < TPB:
                        ak = min(16, TPB - a0)
                        nc.gpsimd.dma_start(
                            g[:, a0:a0 + ak, :],
                            ea[:, b * TPB + a0:b * TPB + a0 + ak, :],
                            accum_op=ALU.add)
                        a0 += ak
                    q = pq.tile([128, TPB, H], bf16, tag="q")
                    nc.vector.tensor_scalar(q[:], g[:], abs_t, 0.0,
                                            ALU.mult, ALU.max)
                    m = pm.tile([128, TPB, 2, H], bf16, tag="m")
                    nc.scalar.activation(m[:, :, 0:1, :], q[:], AF.Exp,
                                         bias=const_col(t_l * MSG_EPS),
                                         scale=sign_t)
                    nc.vector.tensor_mul(m[:, :, 1:2, :], q[:],
                                         m[:, :, 0:1, :])
                    for j in range(TPB):
                        oh = poh.tile([128, BN], bf16, tag="oh")
                        nc.vector.tensor_scalar(
                            oh[:], nvals_sb[:, 0:BN],
                            dstrel_sb[:, b * TPB + j:b * TPB + j + 1],
                            None, ALU.is_equal)
                        nc.tensor.matmul(
                            acc[0:P, :], oh[:], m[:, j, :, :],
                            start=(j == 0), stop=(j == TPB - 1))

                    # ---- node work for this block ----
                    bH = slice(b * H, (b + 1) * H)
                    b2H = slice(b * 2 * H, (b + 1) * 2 * H)
                    dent = sC[0:P, bH]
                    nc.vector.tensor_scalar(dent, acc[0:P, 0:H], abs_t, 1e-20,
                                            ALU.mult, ALU.add)
                    nc.scalar.activation(sD[0:P, bH], dent, AF.Ln)
                    nc.scalar.activation(dent, sD[0:P, bH], AF.Exp, scale=-1.0)
                    nc.vector.tensor_mul(sD[0:P, bH], acc[0:P, H:2 * H], dent)
                    nc.vector.scalar_tensor_tensor(
                        dent, sD[0:P, bH], MSG_EPS, conv_own[0:P, b, :],
                        ALU.add, ALU.add)
                    # h'_b = dent ; transpose -> hpT
                    tp = ptp.tile([128, 128], f32, tag="tp")
                    nc.tensor.transpose(tp[:, 0:P], dent, eye_sb[0:P, 0:P])
                    nc.scalar.copy(hpT[:, b, 0:P], tp[:, 0:P])
                    z = pz.tile([128, 2 * H], f32, tag="z")
                    nc.tensor.matmul(z[0:P, :], hpT[:, b, 0:P],
                                     w1_sb[:, l, :], start=True, stop=False)
                    nc.tensor.matmul(z[0:P, :], ones_sb[0:1, 0:P],
                                     b1_sb[:, l, :], start=False, stop=True)
                    nc.scalar.copy(zs[0:P, b, :], z[0:P, :])
                    ln_block(zs[0:P, b, :], 2 * H, mlpg_sb[0:P, l, :],
                             mlpb_sb[0:P, l, :], sA[0:P, b2H], b,
                             sA[0:P, b2H], sB[0:P, b2H], sA[0:P, b2H])
                    # transposes of u -> uT0, uT1
                    tpa = ptp.tile([128, 128], f32, tag="tp")
                    nc.tensor.transpose(tpa[:, 0:P],
                                        sA[0:P, b * 2 * H:b * 2 * H + H],
                                        eye_sb[0:P, 0:P])
                    nc.scalar.copy(uT0[:, b, 0:P], tpa[:, 0:P])
                    tpb_ = ptp.tile([128, 128], f32, tag="tp")
                    nc.tensor.transpose(tpb_[:, 0:P],
                                        sA[0:P, b * 2 * H + H:(b + 1) * 2 * H],
                                        eye_sb[0:P, 0:P])
                    nc.scalar.copy(uT1[:, b, 0:P], tpb_[:, 0:P])
                    o = po.tile([128, 128], f32, tag="o")
                    nc.tensor.matmul(o[0:P, 0:H], uT0[:, b, 0:P],
                                     w2a_sb[:, l, :], start=True, stop=False)
                    nc.tensor.matmul(o[0:P, 0:H], uT1[:, b, 0:P],
                                     w2b_sb[:, l, :], start=False, stop=False)
                    nc.tensor.matmul(o[0:P, 0:H], ones_sb[0:1, 0:P],
                                     b2_sb[:, l, :], start=False, stop=True)
                    if l == 0:
                        nc.scalar.copy(h_new[0:P, b, :], o[0:P, 0:H])
                    else:
                        nc.vector.tensor_add(h_new[0:P, b, :], o[0:P, 0:H],
                                             h_prev[0:P, b, :])
                    gi = l + 1 if l < L - 1 else 0
                    ln_block(h_new[0:P, b, :], H, lng_sb[0:P, gi, :],
                             lnb_sb[0:P, gi, :], conv_own[0:P, b, :], b,
                             sA[0:P, bH], sD[0:P, bH], sA[0:P, bH])
                    if l < L - 1:
                        hb = 0 if b < 5 else 1
                        nc.gpsimd.dma_start(slab_vs[hb][:, b - 5 * hb, :],
                                            conv_own[0:P, b, :])
                        if b == 4:
                            nc.gpsimd.collective_compute(
                                "AllGather", ALU.bypass,
                                replica_groups=[list(range(NCORES))],
                                ins=[slabs[0].opt()],
                                outs=[tables[l + 1][0][:]],
                            )
                        elif b == NB - 1:
                            nc.gpsimd.collective_compute(
                                "AllGather", ALU.bypass,
                                replica_groups=[list(range(NCORES))],
                                ins=[slabs[1].opt()],
                                outs=[tables[l + 1][1][:]],
                            )

            # ---------------- head ----------------
            pp = po.tile([128, 128], f32, tag="o")
            for b in range(NB):
                nc.tensor.matmul(pp[:, 0:G], conv_own[0:P, b, :],
                                 goh_sb[0:P, b * G:(b + 1) * G],
                                 start=(b == 0), stop=(b == NB - 1))
            nc.scalar.copy(poolT_sb[:], pp[:, 0:G])
            bounce = pdram.tile([128, G], f32)
            nc.sync.dma_start(bounce[:], poolT_sb[:])
            nc.gpsimd.collective_compute(
                "AllReduce", ALU.add,
                replica_groups=[list(range(NCORES))],
                ins=[bounce.opt()], outs=[pool_red[:]],
            )
            nc.sync.dma_start(poolT_sb[:], pool_red[:])
            lg = po.tile([128, 128], f32, tag="o")
            nc.tensor.matmul(lg[0:G, 0:C], poolT_sb[:, 0:G], linW_sb[:],
                             start=True, stop=False)
            nc.tensor.matmul(lg[0:G, 0:C], ones_sb[0:1, 0:G], linb_sb[:],
                             start=False, stop=True)
            nc.scalar.copy(sC[0:G, 0:C], lg[0:G, 0:C])
            nc.sync.dma_start(out_logits[:], sC[0:G, 0:C])
            pl = ptp.tile([128, 128], f32, tag="tp")
            nc.tensor.transpose(pl[0:G, 0:H], poolT_sb[:, 0:G], eye_sb[:])
            nc.scalar.copy(sD[0:G, 0:H], pl[0:G, 0:H])
            nc.sync.dma_start(out_pooled[:], sD[0:G, 0:H])

    # All ACT funcs used here (Relu/Exp/Ln/Copy) live in the
    # natural_log_exp_and_others table set. The load-insertion pass picks the
    # first set containing each func, which alternates exp_and_others /
    # natural_log and thrashes ~2.7us table loads per block. Restrict
    # candidates to the covering set (keeping act_func_set_id positions).
    import concourse.bacc as _bacc_mod
    _orig_tables = _bacc_mod.get_activation_tables

    def _only_nle(arch):
        tabs = _orig_tables(arch)
        return {k: (v if k == "natural_log_exp_and_others" else set())
                for k, v in tabs.items()}

    _bacc_mod.get_activation_tables = _only_nle
    try:
        nc.compile()
    finally:
        _bacc_mod.get_activation_tables = _orig_tables
    return nc


# ----------------------------------------------------------------------------
# host side
# ----------------------------------------------------------------------------

def kernel(**inputs):
    global last_results
    from concourse.bass_utils import run_bass_kernel_spmd

    import ml_dtypes
    bf16_np = ml_dtypes.bfloat16
    f32 = np.float32
    x = np.ascontiguousarray(np.asarray(inputs["x"]), dtype=f32)
    edge_attr = np.ascontiguousarray(np.asarray(inputs["edge_attr"]), dtype=f32)
    ei = np.asarray(inputs["edge_index"]).astype(np.int64)
    batch = np.asarray(inputs["batch"]).astype(np.int64)
    t = np.asarray(inputs["t"], dtype=f32)
    enc_W = np.asarray(inputs["enc_W"], dtype=f32)
    enc_b = np.asarray(inputs["enc_b"], dtype=f32)
    W1 = np.asarray(inputs["W1"], dtype=f32)
    b1 = np.asarray(inputs["b1"], dtype=f32)
    mlp_ln_g = np.asarray(inputs["mlp_ln_g"], dtype=f32)
    mlp_ln_b = np.asarray(inputs["mlp_ln_b"], dtype=f32)
    W2 = np.asarray(inputs["W2"], dtype=f32)
    b2 = np.asarray(inputs["b2"], dtype=f32)
    ln_g = np.asarray(inputs["ln_g"], dtype=f32)
    ln_b = np.asarray(inputs["ln_b"], dtype=f32)
    lin_W = np.asarray(inputs["lin_W"], dtype=f32)
    lin_b = np.asarray(inputs["lin_b"], dtype=f32)

    assert x.shape == (N, H) and edge_attr.shape == (E, H)
    assert ei.shape == (2, E) and batch.shape == (N,)

    src, dst = ei[0], ei[1]
    # src half id: 0 if src is in the first 5 blocks of its owner core
    src_half = ((src % NPC) // (NPC // 2)).astype(np.int64)
    # row of src within its half-table: core*625 + (src_local % 625)
    src_row = (src // NPC) * (NPC // 2) + (src % NPC) % (NPC // 2)
    order = np.lexsort((dst, src_half, dst // BN))
    dsts = dst[order]
    halves = src_half[order]
    rows = src_row[order]
    # per (global block gb, group) counts
    key = (dsts // BN) * 2 + halves
    cntk = np.bincount(key, minlength=2 * (N // BN))
    HTPB = int(np.ceil(cntk.max() / 128))
    HTPB = max(HTPB, 1)
    TPB = 2 * HTPB
    NT = NB * TPB
    E_PAD = NT * 128
    kb = np.searchsorted(key, np.arange(2 * (N // BN) + 1))

    # shared (identical on every core) arrays
    shared = {
        "nvals": np.broadcast_to(np.arange(128, dtype=f32), (128, 128)).astype(bf16_np),
        "eye": np.eye(128, dtype=f32),
        "encW": enc_W.copy(),
        "encb": enc_b.reshape(1, H).copy(),
        "w1": np.ascontiguousarray(W1.transpose(1, 0, 2)),
        "b1": np.ascontiguousarray(b1.reshape(1, L, 2 * H)),
        "w2a": np.ascontiguousarray(W2[:, 0:H, :].transpose(1, 0, 2)),
        "w2b": np.ascontiguousarray(W2[:, H:2 * H, :].transpose(1, 0, 2)),
        "b2": np.ascontiguousarray(b2.reshape(1, L, H)),
        "mlpg": np.ascontiguousarray(
            np.broadcast_to(mlp_ln_g[None], (128, L, 2 * H))),
        "mlpb": np.ascontiguousarray(
            np.broadcast_to(mlp_ln_b[None], (128, L, 2 * H))),
        "lng": np.ascontiguousarray(np.broadcast_to(ln_g[None], (128, L, H))),
        "lnb": np.ascontiguousarray(np.broadcast_to(ln_b[None], (128, L, H))),
        "linW": lin_W.copy(),
        "linb": lin_b.reshape(1, C).copy(),
    }

    xT = np.ascontiguousarray(x.T)
    in_maps = []
    for c in range(NCORES):
        idxg = np.zeros(E_PAD, np.int16)
        drel = np.full(E_PAD, -1.0, f32)
        eap = np.zeros((E_PAD, H), f32)
        for b in range(NB):
            gb = c * NB + b
            for grp in range(2):
                s0 = int(kb[gb * 2 + grp])
                s1 = int(kb[gb * 2 + grp + 1])
                k = s1 - s0
                o0 = (b * TPB + grp * HTPB) * 128
                idxg[o0:o0 + k] = rows[s0:s1].astype(np.int16)
                drel[o0:o0 + k] = (dsts[s0:s1] - gb * BN).astype(f32)
                eap[o0:o0 + k] = edge_attr[order[s0:s1]]
        ea_sh = np.ascontiguousarray(
            eap.reshape(NT, 128, H).transpose(1, 0, 2)).astype(bf16_np)
        drel_sh = np.ascontiguousarray(drel.reshape(NT, 128).T)
        idx_rep = np.ascontiguousarray(
            np.tile(idxg.reshape(E_PAD // 16, 16).T, (8, 1)))
        goh = np.zeros((128, NB * G), f32)
        nloc = np.arange(NPC)
        goh[nloc % BN, (nloc // BN) * G + batch[c * NPC + nloc]] = 1.0
        m = {
            "ea": ea_sh, "idxs": idx_rep, "dstrel": drel_sh,
            "xT": np.ascontiguousarray(xT[:, c * NPC:(c + 1) * NPC]),
            "goh": goh,
        }
        m.update(shared)
        in_maps.append(m)

    key = (TPB, tuple(np.round(t.astype(np.float64), 9).tolist()))
    if key not in _cache:
        _cache[key] = _build_program(TPB, t)
    nc = _cache[key]

    trace = os.environ.get("GNN_TRACE", "0") == "1"
    res = run_bass_kernel_spmd(nc, in_maps, core_ids=list(range(NCORES)),
                               trace=trace)
    last_results = res
    r0 = res.results[0]
    return (np.asarray(r0["out_logits"]), np.asarray(r0["out_pooled"]))


# revision 29
# speedup vs baseline: 1.0042x; 1.0042x over previous
"""Trainium2 Bass kernel for nn_DeepGCN (GENConv softmax-aggr, 4 layers).

Sharding: edges partitioned by destination-node range across 8 cores (each
core owns 1250 consecutive nodes and all edges pointing into them); per layer
the full gather table (node features) is rebuilt with an AllGather collective.

Per 125-node block, softmax aggregation is computed as PE matmuls with
per-tile one-hot dst matrices accumulating [sum(exp), sum(msg*exp)] in PSUM;
h[src] rows are fetched with dma_gather from the DRAM table and edge_attr is
added inline by an accumulating DMA (CCE add).

Self-contained: only needs numpy + the installed concourse/bass stack.
"""

import os
import numpy as np

# ---- problem constants (hardcoded per the task spec) ----
N = 10000
E = 320000
H = 128
L = 4
G = 64            # num graphs
C = 10            # num classes
MSG_EPS = 1e-7
LN_EPS = 1e-5

NCORES = 8
NPC = N // NCORES          # 1250 nodes per core
NB = 10                    # node blocks per core
BN = NPC // NB             # 125 nodes per block
P = BN                     # partition count for node-side ops

_cache = {}
last_results = None        # BassKernelResults of the most recent run (for test.py)


def _build_program(TPB, t_vals):
    import concourse.bacc as bacc
    import concourse.tile as tile
    import concourse.mybir as mybir

    f32 = mybir.dt.float32
    i16 = mybir.dt.int16
    ALU = mybir.AluOpType
    AF = mybir.ActivationFunctionType
    AX = mybir.AxisListType

    NT = NB * TPB              # edge tiles per core
    E_PAD = NT * 128

    nc = bacc.Bacc("TRN2", target_bir_lowering=False, debug=False,
                   num_devices=NCORES,
                   num_swdge_queues=4 if os.environ.get('GNN_QUEUES','4')=='4' else 1)

    # ---------------- dram I/O ----------------
    bf16 = mybir.dt.bfloat16
    ea = nc.dram_tensor("ea", [128, NT, H], bf16, kind="ExternalInput")
    idxs_d = nc.dram_tensor("idxs", [128, E_PAD // 16], i16, kind="ExternalInput")
    dstrel_d = nc.dram_tensor("dstrel", [128, NT], f32, kind="ExternalInput")
    xT_d = nc.dram_tensor("xT", [128, NPC], f32, kind="ExternalInput")
    goh_d = nc.dram_tensor("goh", [128, NB * G], f32, kind="ExternalInput")
    nvals_d = nc.dram_tensor("nvals", [128, 128], bf16, kind="ExternalInput")
    eye_d = nc.dram_tensor("eye", [128, 128], f32, kind="ExternalInput")
    encW_d = nc.dram_tensor("encW", [128, H], f32, kind="ExternalInput")
    encb_d = nc.dram_tensor("encb", [1, H], f32, kind="ExternalInput")
    w1_d = nc.dram_tensor("w1", [128, L, 2 * H], f32, kind="ExternalInput")
    b1_d = nc.dram_tensor("b1", [1, L, 2 * H], f32, kind="ExternalInput")
    w2a_d = nc.dram_tensor("w2a", [128, L, H], f32, kind="ExternalInput")
    w2b_d = nc.dram_tensor("w2b", [128, L, H], f32, kind="ExternalInput")
    b2_d = nc.dram_tensor("b2", [1, L, H], f32, kind="ExternalInput")
    mlpg_d = nc.dram_tensor("mlpg", [128, L, 2 * H], f32, kind="ExternalInput")
    mlpb_d = nc.dram_tensor("mlpb", [128, L, 2 * H], f32, kind="ExternalInput")
    lng_d = nc.dram_tensor("lng", [128, L, H], f32, kind="ExternalInput")
    lnb_d = nc.dram_tensor("lnb", [128, L, H], f32, kind="ExternalInput")
    linW_d = nc.dram_tensor("linW", [128, C], f32, kind="ExternalInput")
    linb_d = nc.dram_tensor("linb", [1, C], f32, kind="ExternalInput")

    out_logits = nc.dram_tensor("out_logits", [G, C], f32, kind="ExternalOutput")
    out_pooled = nc.dram_tensor("out_pooled", [G, H], f32, kind="ExternalOutput")

    tables = [(nc.dram_tensor(f"table{l}a", [N // 2, H], bf16, kind="Internal"),
               nc.dram_tensor(f"table{l}b", [N // 2, H], bf16, kind="Internal"))
              for l in range(L)]
    pool_red = nc.dram_tensor("pool_red", [128, G], f32, kind="Internal")

    # ---------------- sbuf persistents ----------------
    def sbt(name, shape, dtype=f32):
        return nc.alloc_sbuf_tensor(name, list(shape), dtype)

    idx_sb = sbt("idx_sb", [128, E_PAD // 16], i16)
    dstrel_sb = sbt("dstrel_sb", [128, NT])
    nvals_sb = sbt("nvals_sb", [128, 128], bf16)
    eye_sb = sbt("eye_sb", [128, 128])
    ones_sb = sbt("ones_sb", [1, 128])
    onesb_sb = sbt("onesb_sb", [1, 128], bf16)
    goh_sb = sbt("goh_sb", [128, NB * G])
    xT_sb = sbt("xT_sb", [128, NPC])
    encW_sb = sbt("encW_sb", [128, H])
    encb_sb = sbt("encb_sb", [1, H])
    w1_sb = sbt("w1_sb", [128, L, 2 * H])
    b1_sb = sbt("b1_sb", [1, L, 2 * H])
    w2a_sb = sbt("w2a_sb", [128, L, H])
    w2b_sb = sbt("w2b_sb", [128, L, H])
    b2_sb = sbt("b2_sb", [1, L, H])
    mlpg_sb = sbt("mlpg_sb", [128, L, 2 * H])
    mlpb_sb = sbt("mlpb_sb", [128, L, 2 * H])
    lng_sb = sbt("lng_sb", [128, L, H])
    lnb_sb = sbt("lnb_sb", [128, L, H])
    linW_sb = sbt("linW_sb", [128, C])
    linb_sb = sbt("linb_sb", [1, C])

    conv_own = sbt("conv_own", [128, NB, H])     # gather-table slab of own nodes
    h_a = sbt("h_a", [128, NB, H])
    h_b = sbt("h_b", [128, NB, H])
    zs = sbt("zs", [128, NB, 2 * H])             # mm1 output (sbuf copy)
    hpT = sbt("hpT", [128, NB, H])               # h' transposed
    uT0 = sbt("uT0", [128, NB, H])
    uT1 = sbt("uT1", [128, NB, H])
    sA = sbt("sA", [128, NB * 2 * H])            # scratch
    sB = sbt("sB", [128, NB * 2 * H])
    sC = sbt("sC", [128, NB * H])
    sD = sbt("sD", [128, NB * H])
    poolT_sb = sbt("poolT_sb", [128, G])
    st1 = sbt("st1", [128, NB])
    st2 = sbt("st2", [128, NB])
    st3 = sbt("st3", [128, NB])
    st4 = sbt("st4", [128, NB])

    _const_cols = {}

    def const_col(val, parts=128):
        """[parts, 1] SBUF column filled with `val` (for activation bias).

        Must be called inside the TileContext (emits a memset on first use).
        """
        val = float(val)
        if val not in _const_cols:
            t = sbt(f"constc_{len(_const_cols)}", [128, 1])
            nc.vector.memset(t[:], val)
            _const_cols[val] = t
        return _const_cols[val][0:parts, 0:1]

    with tile.TileContext(nc) as tc:
        with (
            tc.tile_pool(name="pg", bufs=int(os.environ.get("GNN_BUFS","2"))) as pg,
            tc.tile_pool(name="pq", bufs=int(os.environ.get("GNN_BUFS","2"))) as pq,
            tc.tile_pool(name="pm", bufs=(int(os.environ.get("GNN_BUFS","2")) if TPB <= 40 else 1)) as pm,
            tc.tile_pool(name="poh", bufs=4) as poh,
            tc.tile_pool(name="pacc", bufs=2, space="PSUM") as pacc,
            tc.tile_pool(name="ptp", bufs=2, space="PSUM") as ptp,
            tc.tile_pool(name="pz", bufs=2, space="PSUM") as pz,
            tc.tile_pool(name="po", bufs=2, space="PSUM") as po,
            tc.tile_pool(name="pdram", bufs=2, space="DRAM") as pdram,
        ):
            # ---------------- load persistents ----------------
            for dst_t, src_t in [
                (idx_sb, idxs_d), (dstrel_sb, dstrel_d), (nvals_sb, nvals_d),
                (eye_sb, eye_d), (goh_sb, goh_d), (xT_sb, xT_d),
                (encW_sb, encW_d), (encb_sb, encb_d),
                (w1_sb, w1_d), (b1_sb, b1_d), (w2a_sb, w2a_d),
                (w2b_sb, w2b_d), (b2_sb, b2_d), (mlpg_sb, mlpg_d),
                (mlpb_sb, mlpb_d), (lng_sb, lng_d), (lnb_sb, lnb_d),
                (linW_sb, linW_d), (linb_sb, linb_d),
            ]:
                nc.sync.dma_start(dst_t[:], src_t[:])
            nc.vector.memset(ones_sb[:], 1.0)
            nc.vector.memset(onesb_sb[:], 1.0)

            def ag_half(table_half, half):
                """AllGather blocks [5*half, 5*half+5) of conv_own."""
                slab = pdram.tile([NPC // 2, H], bf16)
                slab_v = slab[:].rearrange("(b i) h -> i b h", i=BN)
                nc.gpsimd.dma_start(
                    slab_v, conv_own[0:P, 5 * half:5 * half + 5, :])
                nc.gpsimd.collective_compute(
                    "AllGather", ALU.bypass,
                    replica_groups=[list(range(NCORES))],
                    ins=[slab.opt()], outs=[table_half[:]],
                )

            def allgather_slab(table):
                ag_half(table[0], 0)
                ag_half(table[1], 1)

            def emit_ln_relu(src_full, src_blk, F, g_ap, b_ap, dst_full):
                """dst = relu(LN(src) * g + b); src viewed as [P, NB, F]."""
                inv = 1.0 / F
                nf = NB * F
                sqv = sA[0:P, 0:nf]
                nc.vector.tensor_mul(sqv, src_full, src_full)
                nc.vector.reduce_sum(st1[0:P, :], src_full, axis=AX.X)
                nc.vector.reduce_sum(
                    st2[0:P, :],
                    sA[0:P, 0:nf].rearrange("p (b f) -> p b f", f=F),
                    axis=AX.X)
                nc.vector.tensor_scalar(st1[0:P, :], st1[0:P, :], -inv, None,
                                        ALU.mult)
                nc.vector.tensor_mul(st3[0:P, :], st1[0:P, :], st1[0:P, :])
                nc.vector.tensor_scalar(st2[0:P, :], st2[0:P, :], inv, None,
                                        ALU.mult)
                nc.vector.tensor_sub(st4[0:P, :], st2[0:P, :], st3[0:P, :])
                nc.scalar.activation(st2[0:P, :], st4[0:P, :], AF.Ln,
                                     bias=const_col(LN_EPS, P))
                nc.scalar.activation(st3[0:P, :], st2[0:P, :], AF.Exp,
                                     scale=-0.5)
                for b in range(NB):
                    nc.vector.tensor_scalar(
                        sB[0:P, b * F:(b + 1) * F], src_blk(b),
                        st1[0:P, b:b + 1], st3[0:P, b:b + 1],
                        ALU.add, ALU.mult)
                for b in range(NB):
                    nc.vector.tensor_mul(sA[0:P, b * F:(b + 1) * F],
                                         sB[0:P, b * F:(b + 1) * F], g_ap)
                for b in range(NB):
                    nc.vector.tensor_add(sB[0:P, b * F:(b + 1) * F],
                                         sA[0:P, b * F:(b + 1) * F], b_ap)
                nc.scalar.activation(dst_full, sB[0:P, 0:nf], AF.Relu)

            # ---------------- encoder: h0 = x @ encW + encb ----------------
            for b in range(NB):
                ps = po.tile([128, 128], f32, tag="o")
                nc.tensor.matmul(ps[0:P, 0:H], xT_sb[:, b * BN:(b + 1) * BN],
                                 encW_sb[:], start=True, stop=False)
                nc.tensor.matmul(ps[0:P, 0:H], ones_sb[0:1, 0:P],
                                 encb_sb[:], start=False, stop=True)
                nc.scalar.copy(conv_own[0:P, b, :], ps[0:P, 0:H])
            allgather_slab(tables[0])

            # ---------------- layers ----------------
            n_layers = int(os.environ.get("GNN_LAYERS", str(L)))
            h_bufs = [h_a, h_b]
            for l in range(n_layers):
                t_l = float(t_vals[l])
                abs_t = abs(t_l) if t_l != 0.0 else 1e-12
                sign_t = 1.0 if t_l >= 0 else -1.0
                table = tables[l]
                h_new = h_bufs[l % 2]
                h_prev = h_bufs[(l + 1) % 2]

                # ---- edge + per-block node phase (pipelined) ----
                GCH = 8
                slab_a = pdram.tile([NPC // 2, H], bf16, tag="slab_a")
                slab_b = pdram.tile([NPC // 2, H], bf16, tag="slab_b")
                slabs = [slab_a, slab_b]
                slab_vs = [slab_a[:].rearrange("(b i) h -> i b h", i=BN),
                           slab_b[:].rearrange("(b i) h -> i b h", i=BN)]

                def ln_block(src_ap, F, g_ap, b_ap, dst_ap, b, sq_ap, u_ap,
                             v_ap):
                    """dst = relu(LN(src)*g+b) for one 125-node block.

                    src/dst: [P, F] APs (SBUF). Uses st1..st4 column b.
                    """
                    inv = 1.0 / F
                    c1 = st1[0:P, b:b + 1]
                    c2 = st2[0:P, b:b + 1]
                    c3 = st3[0:P, b:b + 1]
                    c4 = st4[0:P, b:b + 1]
                    nc.vector.tensor_mul(sq_ap, src_ap, src_ap)
                    nc.vector.reduce_sum(c1, src_ap, axis=AX.X)
                    nc.vector.reduce_sum(c2, sq_ap, axis=AX.X)
                    nc.vector.tensor_scalar(c1, c1, -inv, None, ALU.mult)
                    nc.vector.tensor_mul(c3, c1, c1)
                    nc.vector.tensor_scalar(c2, c2, inv, None, ALU.mult)
                    nc.vector.tensor_sub(c4, c2, c3)
                    nc.scalar.activation(c2, c4, AF.Ln, bias=const_col(LN_EPS, P))
                    nc.scalar.activation(c3, c2, AF.Exp, scale=-0.5)
                    nc.vector.tensor_scalar(u_ap, src_ap, c1, c3,
                                            ALU.add, ALU.mult)
                    nc.vector.tensor_mul(v_ap, u_ap, g_ap)
                    nc.vector.tensor_add(u_ap, v_ap, b_ap)
                    nc.scalar.activation(dst_ap, u_ap, AF.Relu)

                HTPB = TPB // 2
                for b in range(NB):
                    acc = pacc.tile([128, 2 * H], f32, tag="acc")
                    g = pg.tile([128, TPB, H], bf16, tag="g")
                    qn = 0
                    for grp in range(2):
                        done = 0
                        while done < HTPB:
                            ck = min(GCH, HTPB - done)
                            t0 = b * TPB + grp * HTPB + done
                            nc.gpsimd.dma_gather(
                                g[:, grp * HTPB + done:
                                  grp * HTPB + done + ck, :],
                                table[grp][:],
                                idx_sb[:, t0 * 8:(t0 + ck) * 8],
                                ck * 128, ck * 128, H,
                                queue_num=(qn % 4) if os.environ.get('GNN_QUEUES','4')=='4' else 0)
                            qn += 1
                            done += ck
                    # CCE accumulate caps at 2048 elements/partition per
                    # transfer -> split into <=16-tile pieces.
                    a0 = 0
                    while a0 < TPB:
                        ak = min(16, TPB - a0)
                        nc.gpsimd.dma_start(
                            g[:, a0:a0 + ak, :],
                            ea[:, b * TPB + a0:b * TPB + a0 + ak, :],
                            accum_op=ALU.add)
                        a0 += ak
                    q = pq.tile([128, TPB, H], bf16, tag="q")
                    nc.vector.tensor_scalar(q[:], g[:], abs_t, 0.0,
                                            ALU.mult, ALU.max)
                    m = pm.tile([128, TPB, 2, H], bf16, tag="m")
                    nc.scalar.activation(m[:, :, 0:1, :], q[:], AF.Exp,
                                         bias=const_col(t_l * MSG_EPS),
                                         scale=sign_t)
                    nc.vector.tensor_mul(m[:, :, 1:2, :], q[:],
                                         m[:, :, 0:1, :])
                    OHW = BN + 1  # 126: even free dim -> DVE 4x mode
                    for j in range(TPB):
                        oh = poh.tile([128, OHW], bf16, tag="oh")
                        nc.vector.tensor_scalar(
                            oh[:], nvals_sb[:, 0:OHW],
                            dstrel_sb[:, b * TPB + j:b * TPB + j + 1],
                            None, ALU.is_equal)
                        nc.tensor.matmul(
                            acc[0:OHW, :], oh[:], m[:, j, :, :],
                            start=(j == 0), stop=(j == TPB - 1))

                    # ---- node work for this block ----
                    bH = slice(b * H, (b + 1) * H)
                    b2H = slice(b * 2 * H, (b + 1) * 2 * H)
                    dent = sC[0:P, bH]
                    nc.vector.tensor_scalar(dent, acc[0:P, 0:H], abs_t, 1e-20,
                                            ALU.mult, ALU.add)
                    nc.scalar.activation(sD[0:P, bH], dent, AF.Ln)
                    nc.scalar.activation(dent, sD[0:P, bH], AF.Exp, scale=-1.0)
                    nc.vector.tensor_mul(sD[0:P, bH], acc[0:P, H:2 * H], dent)
                    nc.vector.scalar_tensor_tensor(
                        dent, sD[0:P, bH], MSG_EPS, conv_own[0:P, b, :],
                        ALU.add, ALU.add)
                    # h'_b = dent ; transpose -> hpT
                    tp = ptp.tile([128, 128], f32, tag="tp")
                    nc.tensor.transpose(tp[:, 0:P], dent, eye_sb[0:P, 0:P])
                    nc.scalar.copy(hpT[:, b, 0:P], tp[:, 0:P])
                    z = pz.tile([128, 2 * H], f32, tag="z")
                    nc.tensor.matmul(z[0:P, :], hpT[:, b, 0:P],
                                     w1_sb[:, l, :], start=True, stop=False)
                    nc.tensor.matmul(z[0:P, :], ones_sb[0:1, 0:P],
                                     b1_sb[:, l, :], start=False, stop=True)
                    nc.scalar.copy(zs[0:P, b, :], z[0:P, :])
                    ln_block(zs[0:P, b, :], 2 * H, mlpg_sb[0:P, l, :],
                             mlpb_sb[0:P, l, :], sA[0:P, b2H], b,
                             sA[0:P, b2H], sB[0:P, b2H], sA[0:P, b2H])
                    # transposes of u -> uT0, uT1
                    tpa = ptp.tile([128, 128], f32, tag="tp")
                    nc.tensor.transpose(tpa[:, 0:P],
                                        sA[0:P, b * 2 * H:b * 2 * H + H],
                                        eye_sb[0:P, 0:P])
                    nc.scalar.copy(uT0[:, b, 0:P], tpa[:, 0:P])
                    tpb_ = ptp.tile([128, 128], f32, tag="tp")
                    nc.tensor.transpose(tpb_[:, 0:P],
                                        sA[0:P, b * 2 * H + H:(b + 1) * 2 * H],
                                        eye_sb[0:P, 0:P])
                    nc.scalar.copy(uT1[:, b, 0:P], tpb_[:, 0:P])
                    o = po.tile([128, 128], f32, tag="o")
                    nc.tensor.matmul(o[0:P, 0:H], uT0[:, b, 0:P],
                                     w2a_sb[:, l, :], start=True, stop=False)
                    nc.tensor.matmul(o[0:P, 0:H], uT1[:, b, 0:P],
                                     w2b_sb[:, l, :], start=False, stop=False)
                    nc.tensor.matmul(o[0:P, 0:H], ones_sb[0:1, 0:P],
                                     b2_sb[:, l, :], start=False, stop=True)
                    if l == 0:
                        nc.scalar.copy(h_new[0:P, b, :], o[0:P, 0:H])
                    else:
                        nc.vector.tensor_add(h_new[0:P, b, :], o[0:P, 0:H],
                                             h_prev[0:P, b, :])
                    gi = l + 1 if l < L - 1 else 0
                    ln_block(h_new[0:P, b, :], H, lng_sb[0:P, gi, :],
                             lnb_sb[0:P, gi, :], conv_own[0:P, b, :], b,
                             sA[0:P, bH], sD[0:P, bH], sA[0:P, bH])
                    if l < L - 1:
                        hb = 0 if b < 5 else 1
                        if b == 4 or b == NB - 1:
                            nc.gpsimd.dma_start(
                                slab_vs[hb][:, :, :],
                                conv_own[0:P, 5 * hb:5 * hb + 5, :])
                        if b == 4:
                            nc.gpsimd.collective_compute(
                                "AllGather", ALU.bypass,
                                replica_groups=[list(range(NCORES))],
                                ins=[slabs[0].opt()],
                                outs=[tables[l + 1][0][:]],
                            )
                        elif b == NB - 1:
                            nc.gpsimd.collective_compute(
                                "AllGather", ALU.bypass,
                                replica_groups=[list(range(NCORES))],
                                ins=[slabs[1].opt()],
                                outs=[tables[l + 1][1][:]],
                            )

            # ---------------- head ----------------
            pp = po.tile([128, 128], f32, tag="o")
            for b in range(NB):
                nc.tensor.matmul(pp[:, 0:G], conv_own[0:P, b, :],
                                 goh_sb[0:P, b * G:(b + 1) * G],
                                 start=(b == 0), stop=(b == NB - 1))
            nc.scalar.copy(poolT_sb[:], pp[:, 0:G])
            bounce = pdram.tile([128, G], f32)
            nc.sync.dma_start(bounce[:], poolT_sb[:])
            nc.gpsimd.collective_compute(
                "AllReduce", ALU.add,
                replica_groups=[list(range(NCORES))],
                ins=[bounce.opt()], outs=[pool_red[:]],
            )
            nc.sync.dma_start(poolT_sb[:], pool_red[:])
            lg = po.tile([128, 128], f32, tag="o")
            nc.tensor.matmul(lg[0:G, 0:C], poolT_sb[:, 0:G], linW_sb[:],
                             start=True, stop=False)
            nc.tensor.matmul(lg[0:G, 0:C], ones_sb[0:1, 0:G], linb_sb[:],
                             start=False, stop=True)
            nc.scalar.copy(sC[0:G, 0:C], lg[0:G, 0:C])
            nc.sync.dma_start(out_logits[:], sC[0:G, 0:C])
            pl = ptp.tile([128, 128], f32, tag="tp")
            nc.tensor.transpose(pl[0:G, 0:H], poolT_sb[:, 0:G], eye_sb[:])
            nc.scalar.copy(sD[0:G, 0:H], pl[0:G, 0:H])
            nc.sync.dma_start(out_pooled[:], sD[0:G, 0:H])

    # All ACT funcs used here (Relu/Exp/Ln/Copy) live in the
    # natural_log_exp_and_others table set. The load-insertion pass picks the
    # first set containing each func, which alternates exp_and_others /
    # natural_log and thrashes ~2.7us table loads per block. Restrict
    # candidates to the covering set (keeping act_func_set_id positions).
    import concourse.bacc as _bacc_mod
    _orig_tables = _bacc_mod.get_activation_tables

    def _only_nle(arch):
        tabs = _orig_tables(arch)
        return {k: (v if k == "natural_log_exp_and_others" else set())
                for k, v in tabs.items()}

    _bacc_mod.get_activation_tables = _only_nle
    try:
        nc.compile()
    finally:
        _bacc_mod.get_activation_tables = _orig_tables
    return nc


# ----------------------------------------------------------------------------
# host side
# ----------------------------------------------------------------------------

def kernel(**inputs):
    global last_results
    from concourse.bass_utils import run_bass_kernel_spmd

    import ml_dtypes
    bf16_np = ml_dtypes.bfloat16
    f32 = np.float32
    x = np.ascontiguousarray(np.asarray(inputs["x"]), dtype=f32)
    edge_attr = np.ascontiguousarray(np.asarray(inputs["edge_attr"]), dtype=f32)
    ei = np.asarray(inputs["edge_index"]).astype(np.int64)
    batch = np.asarray(inputs["batch"]).astype(np.int64)
    t = np.asarray(inputs["t"], dtype=f32)
    enc_W = np.asarray(inputs["enc_W"], dtype=f32)
    enc_b = np.asarray(inputs["enc_b"], dtype=f32)
    W1 = np.asarray(inputs["W1"], dtype=f32)
    b1 = np.asarray(inputs["b1"], dtype=f32)
    mlp_ln_g = np.asarray(inputs["mlp_ln_g"], dtype=f32)
    mlp_ln_b = np.asarray(inputs["mlp_ln_b"], dtype=f32)
    W2 = np.asarray(inputs["W2"], dtype=f32)
    b2 = np.asarray(inputs["b2"], dtype=f32)
    ln_g = np.asarray(inputs["ln_g"], dtype=f32)
    ln_b = np.asarray(inputs["ln_b"], dtype=f32)
    lin_W = np.asarray(inputs["lin_W"], dtype=f32)
    lin_b = np.asarray(inputs["lin_b"], dtype=f32)

    assert x.shape == (N, H) and edge_attr.shape == (E, H)
    assert ei.shape == (2, E) and batch.shape == (N,)

    src, dst = ei[0], ei[1]
    # src half id: 0 if src is in the first 5 blocks of its owner core
    src_half = ((src % NPC) // (NPC // 2)).astype(np.int64)
    # row of src within its half-table: core*625 + (src_local % 625)
    src_row = (src // NPC) * (NPC // 2) + (src % NPC) % (NPC // 2)
    order = np.lexsort((dst, src_half, dst // BN))
    dsts = dst[order]
    halves = src_half[order]
    rows = src_row[order]
    # per (global block gb, group) counts
    key = (dsts // BN) * 2 + halves
    cntk = np.bincount(key, minlength=2 * (N // BN))
    HTPB = int(np.ceil(cntk.max() / 128))
    HTPB = max(HTPB, 1)
    TPB = 2 * HTPB
    NT = NB * TPB
    E_PAD = NT * 128
    kb = np.searchsorted(key, np.arange(2 * (N // BN) + 1))

    # shared (identical on every core) arrays
    shared = {
        "nvals": np.broadcast_to(np.arange(128, dtype=f32), (128, 128)).astype(bf16_np),
        "eye": np.eye(128, dtype=f32),
        "encW": enc_W.copy(),
        "encb": enc_b.reshape(1, H).copy(),
        "w1": np.ascontiguousarray(W1.transpose(1, 0, 2)),
        "b1": np.ascontiguousarray(b1.reshape(1, L, 2 * H)),
        "w2a": np.ascontiguousarray(W2[:, 0:H, :].transpose(1, 0, 2)),
        "w2b": np.ascontiguousarray(W2[:, H:2 * H, :].transpose(1, 0, 2)),
        "b2": np.ascontiguousarray(b2.reshape(1, L, H)),
        "mlpg": np.ascontiguousarray(
            np.broadcast_to(mlp_ln_g[None], (128, L, 2 * H))),
        "mlpb": np.ascontiguousarray(
            np.broadcast_to(mlp_ln_b[None], (128, L, 2 * H))),
        "lng": np.ascontiguousarray(np.broadcast_to(ln_g[None], (128, L, H))),
        "lnb": np.ascontiguousarray(np.broadcast_to(ln_b[None], (128, L, H))),
        "linW": lin_W.copy(),
        "linb": lin_b.reshape(1, C).copy(),
    }

    xT = np.ascontiguousarray(x.T)
    in_maps = []
    for c in range(NCORES):
        idxg = np.zeros(E_PAD, np.int16)
        drel = np.full(E_PAD, -1.0, f32)
        eap = np.zeros((E_PAD, H), f32)
        for b in range(NB):
            gb = c * NB + b
            for grp in range(2):
                s0 = int(kb[gb * 2 + grp])
                s1 = int(kb[gb * 2 + grp + 1])
                k = s1 - s0
                o0 = (b * TPB + grp * HTPB) * 128
                idxg[o0:o0 + k] = rows[s0:s1].astype(np.int16)
                drel[o0:o0 + k] = (dsts[s0:s1] - gb * BN).astype(f32)
                eap[o0:o0 + k] = edge_attr[order[s0:s1]]
        ea_sh = np.ascontiguousarray(
            eap.reshape(NT, 128, H).transpose(1, 0, 2)).astype(bf16_np)
        drel_sh = np.ascontiguousarray(drel.reshape(NT, 128).T)
        idx_rep = np.ascontiguousarray(
            np.tile(idxg.reshape(E_PAD // 16, 16).T, (8, 1)))
        goh = np.zeros((128, NB * G), f32)
        nloc = np.arange(NPC)
        goh[nloc % BN, (nloc // BN) * G + batch[c * NPC + nloc]] = 1.0
        m = {
            "ea": ea_sh, "idxs": idx_rep, "dstrel": drel_sh,
            "xT": np.ascontiguousarray(xT[:, c * NPC:(c + 1) * NPC]),
            "goh": goh,
        }
        m.update(shared)
        in_maps.append(m)

    key = (TPB, tuple(np.round(t.astype(np.float64), 9).tolist()))
    if key not in _cache:
        _cache[key] = _build_program(TPB, t)
    nc = _cache[key]

    trace = os.environ.get("GNN_TRACE", "0") == "1"
    res = run_bass_kernel_spmd(nc, in_maps, core_ids=list(range(NCORES)),
                               trace=trace)
    last_results = res
    r0 = res.results[0]
    return (np.asarray(r0["out_logits"]), np.asarray(r0["out_pooled"]))


# revision 33
# speedup vs baseline: 1.1178x; 1.1131x over previous
"""Trainium2 Bass kernel for nn_DeepGCN (GENConv softmax-aggr, 4 layers).

Sharding: edges partitioned by destination-node range across 8 cores (each
core owns 1250 consecutive nodes and all edges pointing into them); per layer
the full gather table (node features) is rebuilt with an AllGather collective.

Per 125-node block, softmax aggregation is computed as PE matmuls with
per-tile one-hot dst matrices accumulating [sum(exp), sum(msg*exp)] in PSUM;
h[src] rows are fetched with dma_gather from the DRAM table and edge_attr is
added inline by an accumulating DMA (CCE add).

Self-contained: only needs numpy + the installed concourse/bass stack.
"""

import os
import numpy as np

# ---- problem constants (hardcoded per the task spec) ----
N = 10000
E = 320000
H = 128
L = 4
G = 64            # num graphs
C = 10            # num classes
MSG_EPS = 1e-7
LN_EPS = 1e-5

NCORES = 8
NPC = N // NCORES          # 1250 nodes per core
NB = 10                    # node blocks per core
BN = NPC // NB             # 125 nodes per block
P = BN                     # partition count for node-side ops

_cache = {}
last_results = None        # BassKernelResults of the most recent run (for test.py)


def _build_program(TPB, t_vals):
    import concourse.bacc as bacc
    import concourse.tile as tile
    import concourse.mybir as mybir

    f32 = mybir.dt.float32
    i16 = mybir.dt.int16
    ALU = mybir.AluOpType
    AF = mybir.ActivationFunctionType
    AX = mybir.AxisListType

    NT = NB * TPB              # edge tiles per core
    E_PAD = NT * 128

    nc = bacc.Bacc("TRN2", target_bir_lowering=False, debug=False,
                   num_devices=NCORES,
                   num_swdge_queues=4 if os.environ.get('GNN_QUEUES','4')=='4' else 1)

    # ---------------- dram I/O ----------------
    bf16 = mybir.dt.bfloat16
    ea = nc.dram_tensor("ea", [128, NT, H], bf16, kind="ExternalInput")
    idxs_d = nc.dram_tensor("idxs", [128, E_PAD // 16], i16, kind="ExternalInput")
    dstrel_d = nc.dram_tensor("dstrel", [128, NT], f32, kind="ExternalInput")
    xT_d = nc.dram_tensor("xT", [128, NPC], f32, kind="ExternalInput")
    goh_d = nc.dram_tensor("goh", [128, NB * G], f32, kind="ExternalInput")
    nvals_d = nc.dram_tensor("nvals", [128, 128], bf16, kind="ExternalInput")
    eye_d = nc.dram_tensor("eye", [128, 128], f32, kind="ExternalInput")
    encW_d = nc.dram_tensor("encW", [128, H], f32, kind="ExternalInput")
    encb_d = nc.dram_tensor("encb", [1, H], f32, kind="ExternalInput")
    w1_d = nc.dram_tensor("w1", [128, L, 2 * H], f32, kind="ExternalInput")
    b1_d = nc.dram_tensor("b1", [1, L, 2 * H], f32, kind="ExternalInput")
    w2a_d = nc.dram_tensor("w2a", [128, L, H], f32, kind="ExternalInput")
    w2b_d = nc.dram_tensor("w2b", [128, L, H], f32, kind="ExternalInput")
    b2_d = nc.dram_tensor("b2", [1, L, H], f32, kind="ExternalInput")
    mlpg_d = nc.dram_tensor("mlpg", [128, L, 2 * H], f32, kind="ExternalInput")
    mlpb_d = nc.dram_tensor("mlpb", [128, L, 2 * H], f32, kind="ExternalInput")
    lng_d = nc.dram_tensor("lng", [128, L, H], f32, kind="ExternalInput")
    lnb_d = nc.dram_tensor("lnb", [128, L, H], f32, kind="ExternalInput")
    linW_d = nc.dram_tensor("linW", [128, C], f32, kind="ExternalInput")
    linb_d = nc.dram_tensor("linb", [1, C], f32, kind="ExternalInput")

    out_logits = nc.dram_tensor("out_logits", [G, C], f32, kind="ExternalOutput")
    out_pooled = nc.dram_tensor("out_pooled", [G, H], f32, kind="ExternalOutput")

    tables = [(nc.dram_tensor(f"table{l}a", [N // 2, H], bf16, kind="Internal"),
               nc.dram_tensor(f"table{l}b", [N // 2, H], bf16, kind="Internal"))
              for l in range(L)]
    pool_red = nc.dram_tensor("pool_red", [128, G], f32, kind="Internal")

    # ---------------- sbuf persistents ----------------
    def sbt(name, shape, dtype=f32):
        return nc.alloc_sbuf_tensor(name, list(shape), dtype)

    idx_sb = sbt("idx_sb", [128, E_PAD // 16], i16)
    dstrel_sb = sbt("dstrel_sb", [128, NT])
    nvals_sb = sbt("nvals_sb", [128, 128], bf16)
    eye_sb = sbt("eye_sb", [128, 128])
    ones_sb = sbt("ones_sb", [1, 128])
    onesb_sb = sbt("onesb_sb", [1, 128], bf16)
    goh_sb = sbt("goh_sb", [128, NB * G])
    xT_sb = sbt("xT_sb", [128, NPC])
    encW_sb = sbt("encW_sb", [128, H])
    encb_sb = sbt("encb_sb", [1, H])
    w1_sb = sbt("w1_sb", [128, L, 2 * H])
    b1_sb = sbt("b1_sb", [1, L, 2 * H])
    w2a_sb = sbt("w2a_sb", [128, L, H])
    w2b_sb = sbt("w2b_sb", [128, L, H])
    b2_sb = sbt("b2_sb", [1, L, H])
    mlpg_sb = sbt("mlpg_sb", [128, L, 2 * H])
    mlpb_sb = sbt("mlpb_sb", [128, L, 2 * H])
    lng_sb = sbt("lng_sb", [128, L, H])
    lnb_sb = sbt("lnb_sb", [128, L, H])
    linW_sb = sbt("linW_sb", [128, C])
    linb_sb = sbt("linb_sb", [1, C])

    conv_own = sbt("conv_own", [128, NB, H])     # gather-table slab of own nodes
    h_a = sbt("h_a", [128, NB, H])
    h_b = sbt("h_b", [128, NB, H])
    zs = sbt("zs", [128, NB, 2 * H])             # mm1 output (sbuf copy)
    hpT = sbt("hpT", [128, NB, H])               # h' transposed
    uT0 = sbt("uT0", [128, NB, H])
    uT1 = sbt("uT1", [128, NB, H])
    sA = sbt("sA", [128, NB * 2 * H])            # scratch
    sB = sbt("sB", [128, NB * 2 * H])
    sC = sbt("sC", [128, NB * H])
    sD = sbt("sD", [128, NB * H])
    poolT_sb = sbt("poolT_sb", [128, G])
    st1 = sbt("st1", [128, NB])
    st2 = sbt("st2", [128, NB])
    st3 = sbt("st3", [128, NB])
    st4 = sbt("st4", [128, NB])

    _const_cols = {}

    def const_col(val, parts=128):
        """[parts, 1] SBUF column filled with `val` (for activation bias).

        Must be called inside the TileContext (emits a memset on first use).
        """
        val = float(val)
        if val not in _const_cols:
            t = sbt(f"constc_{len(_const_cols)}", [128, 1])
            nc.vector.memset(t[:], val)
            _const_cols[val] = t
        return _const_cols[val][0:parts, 0:1]

    with tile.TileContext(nc) as tc:
        with (
            tc.tile_pool(name="pg", bufs=int(os.environ.get("GNN_BUFS","2"))) as pg,
            tc.tile_pool(name="pq", bufs=int(os.environ.get("GNN_BUFS","2"))) as pq,
            tc.tile_pool(name="pm", bufs=(int(os.environ.get("GNN_BUFS","2")) if TPB <= 40 else 1)) as pm,
            tc.tile_pool(name="poh", bufs=16) as poh,
            tc.tile_pool(name="pacc", bufs=2, space="PSUM") as pacc,
            tc.tile_pool(name="ptp", bufs=2, space="PSUM") as ptp,
            tc.tile_pool(name="pz", bufs=2, space="PSUM") as pz,
            tc.tile_pool(name="po", bufs=2, space="PSUM") as po,
            tc.tile_pool(name="pdram", bufs=2, space="DRAM") as pdram,
        ):
            # ---------------- load persistents ----------------
            for dst_t, src_t in [
                (idx_sb, idxs_d), (dstrel_sb, dstrel_d), (nvals_sb, nvals_d),
                (eye_sb, eye_d), (goh_sb, goh_d), (xT_sb, xT_d),
                (encW_sb, encW_d), (encb_sb, encb_d),
                (w1_sb, w1_d), (b1_sb, b1_d), (w2a_sb, w2a_d),
                (w2b_sb, w2b_d), (b2_sb, b2_d), (mlpg_sb, mlpg_d),
                (mlpb_sb, mlpb_d), (lng_sb, lng_d), (lnb_sb, lnb_d),
                (linW_sb, linW_d), (linb_sb, linb_d),
            ]:
                nc.sync.dma_start(dst_t[:], src_t[:])
            nc.vector.memset(ones_sb[:], 1.0)
            nc.vector.memset(onesb_sb[:], 1.0)

            def ag_half(table_half, half):
                """AllGather blocks [5*half, 5*half+5) of conv_own."""
                slab = pdram.tile([NPC // 2, H], bf16)
                slab_v = slab[:].rearrange("(b i) h -> i b h", i=BN)
                nc.gpsimd.dma_start(
                    slab_v, conv_own[0:P, 5 * half:5 * half + 5, :])
                nc.gpsimd.collective_compute(
                    "AllGather", ALU.bypass,
                    replica_groups=[list(range(NCORES))],
                    ins=[slab.opt()], outs=[table_half[:]],
                )

            def allgather_slab(table):
                ag_half(table[0], 0)
                ag_half(table[1], 1)

            def emit_ln_relu(src_full, src_blk, F, g_ap, b_ap, dst_full):
                """dst = relu(LN(src) * g + b); src viewed as [P, NB, F]."""
                inv = 1.0 / F
                nf = NB * F
                sqv = sA[0:P, 0:nf]
                nc.vector.tensor_mul(sqv, src_full, src_full)
                nc.vector.reduce_sum(st1[0:P, :], src_full, axis=AX.X)
                nc.vector.reduce_sum(
                    st2[0:P, :],
                    sA[0:P, 0:nf].rearrange("p (b f) -> p b f", f=F),
                    axis=AX.X)
                nc.vector.tensor_scalar(st1[0:P, :], st1[0:P, :], -inv, None,
                                        ALU.mult)
                nc.vector.tensor_mul(st3[0:P, :], st1[0:P, :], st1[0:P, :])
                nc.vector.tensor_scalar(st2[0:P, :], st2[0:P, :], inv, None,
                                        ALU.mult)
                nc.vector.tensor_sub(st4[0:P, :], st2[0:P, :], st3[0:P, :])
                nc.scalar.activation(st2[0:P, :], st4[0:P, :], AF.Ln,
                                     bias=const_col(LN_EPS, P))
                nc.scalar.activation(st3[0:P, :], st2[0:P, :], AF.Exp,
                                     scale=-0.5)
                for b in range(NB):
                    nc.vector.tensor_scalar(
                        sB[0:P, b * F:(b + 1) * F], src_blk(b),
                        st1[0:P, b:b + 1], st3[0:P, b:b + 1],
                        ALU.add, ALU.mult)
                for b in range(NB):
                    nc.vector.tensor_mul(sA[0:P, b * F:(b + 1) * F],
                                         sB[0:P, b * F:(b + 1) * F], g_ap)
                for b in range(NB):
                    nc.vector.tensor_add(sB[0:P, b * F:(b + 1) * F],
                                         sA[0:P, b * F:(b + 1) * F], b_ap)
                nc.scalar.activation(dst_full, sB[0:P, 0:nf], AF.Relu)

            # ---------------- encoder: h0 = x @ encW + encb ----------------
            for b in range(NB):
                ps = po.tile([128, 128], f32, tag="o")
                nc.tensor.matmul(ps[0:P, 0:H], xT_sb[:, b * BN:(b + 1) * BN],
                                 encW_sb[:], start=True, stop=False)
                nc.tensor.matmul(ps[0:P, 0:H], ones_sb[0:1, 0:P],
                                 encb_sb[:], start=False, stop=True)
                nc.scalar.copy(conv_own[0:P, b, :], ps[0:P, 0:H])
            allgather_slab(tables[0])

            # ---------------- layers ----------------
            n_layers = int(os.environ.get("GNN_LAYERS", str(L)))
            h_bufs = [h_a, h_b]
            for l in range(n_layers):
                t_l = float(t_vals[l])
                abs_t = abs(t_l) if t_l != 0.0 else 1e-12
                sign_t = 1.0 if t_l >= 0 else -1.0
                table = tables[l]
                h_new = h_bufs[l % 2]
                h_prev = h_bufs[(l + 1) % 2]

                # ---- edge + per-block node phase (pipelined) ----
                GCH = 8
                slab_a = pdram.tile([NPC // 2, H], bf16, tag="slab_a")
                slab_b = pdram.tile([NPC // 2, H], bf16, tag="slab_b")
                slabs = [slab_a, slab_b]
                slab_vs = [slab_a[:].rearrange("(b i) h -> i b h", i=BN),
                           slab_b[:].rearrange("(b i) h -> i b h", i=BN)]

                def ln_block(src_ap, F, g_ap, b_ap, dst_ap, b, sq_ap, u_ap,
                             v_ap):
                    """dst = relu(LN(src)*g+b) for one 125-node block.

                    src/dst: [P, F] APs (SBUF). Uses st1..st4 column b.
                    """
                    inv = 1.0 / F
                    c1 = st1[0:P, b:b + 1]
                    c2 = st2[0:P, b:b + 1]
                    c3 = st3[0:P, b:b + 1]
                    c4 = st4[0:P, b:b + 1]
                    nc.vector.tensor_mul(sq_ap, src_ap, src_ap)
                    nc.vector.reduce_sum(c1, src_ap, axis=AX.X)
                    nc.vector.reduce_sum(c2, sq_ap, axis=AX.X)
                    nc.vector.tensor_scalar(c1, c1, -inv, None, ALU.mult)
                    nc.vector.tensor_mul(c3, c1, c1)
                    nc.vector.tensor_scalar(c2, c2, inv, None, ALU.mult)
                    nc.vector.tensor_sub(c4, c2, c3)
                    nc.scalar.activation(c2, c4, AF.Ln, bias=const_col(LN_EPS, P))
                    nc.scalar.activation(c3, c2, AF.Exp, scale=-0.5)
                    nc.vector.tensor_scalar(u_ap, src_ap, c1, c3,
                                            ALU.add, ALU.mult)
                    nc.vector.tensor_mul(v_ap, u_ap, g_ap)
                    nc.vector.tensor_add(u_ap, v_ap, b_ap)
                    nc.scalar.activation(dst_ap, u_ap, AF.Relu)

                HTPB = TPB // 2
                for b in range(NB):
                    acc = pacc.tile([128, 2 * H], f32, tag="acc")
                    g = pg.tile([128, TPB, H], bf16, tag="g")
                    qn = 0
                    for grp in range(2):
                        done = 0
                        while done < HTPB:
                            ck = min(GCH, HTPB - done)
                            t0 = b * TPB + grp * HTPB + done
                            nc.gpsimd.dma_gather(
                                g[:, grp * HTPB + done:
                                  grp * HTPB + done + ck, :],
                                table[grp][:],
                                idx_sb[:, t0 * 8:(t0 + ck) * 8],
                                ck * 128, ck * 128, H,
                                queue_num=(qn % 4) if os.environ.get('GNN_QUEUES','4')=='4' else 0)
                            qn += 1
                            done += ck
                    # CCE accumulate caps at 2048 elements/partition per
                    # transfer -> split into <=16-tile pieces.
                    a0 = 0
                    while a0 < TPB:
                        ak = min(16, TPB - a0)
                        nc.gpsimd.dma_start(
                            g[:, a0:a0 + ak, :],
                            ea[:, b * TPB + a0:b * TPB + a0 + ak, :],
                            accum_op=ALU.add)
                        a0 += ak
                    q = pq.tile([128, TPB, H], bf16, tag="q")
                    nc.vector.tensor_scalar(q[:], g[:], abs_t, 0.0,
                                            ALU.mult, ALU.max)
                    m = pm.tile([128, TPB, 2, H], bf16, tag="m")
                    nc.scalar.activation(m[:, :, 0:1, :], q[:], AF.Exp,
                                         bias=const_col(t_l * MSG_EPS),
                                         scale=sign_t)
                    nc.vector.tensor_mul(m[:, :, 1:2, :], q[:],
                                         m[:, :, 0:1, :])
                    OHW = BN + 1  # 126: even free dim -> DVE 4x mode
                    for j in range(TPB):
                        oh = poh.tile([128, OHW], bf16, tag="oh")
                        nc.vector.tensor_scalar(
                            oh[:], nvals_sb[:, 0:OHW],
                            dstrel_sb[:, b * TPB + j:b * TPB + j + 1],
                            None, ALU.is_equal)
                        nc.tensor.matmul(
                            acc[0:OHW, :], oh[:], m[:, j, :, :],
                            start=(j == 0), stop=(j == TPB - 1))

                    # ---- node work for this block ----
                    bH = slice(b * H, (b + 1) * H)
                    b2H = slice(b * 2 * H, (b + 1) * 2 * H)
                    dent = sC[0:P, bH]
                    nc.vector.tensor_scalar(dent, acc[0:P, 0:H], abs_t, 1e-20,
                                            ALU.mult, ALU.add)
                    nc.scalar.activation(sD[0:P, bH], dent, AF.Ln)
                    nc.scalar.activation(dent, sD[0:P, bH], AF.Exp, scale=-1.0)
                    nc.vector.tensor_mul(sD[0:P, bH], acc[0:P, H:2 * H], dent)
                    nc.vector.scalar_tensor_tensor(
                        dent, sD[0:P, bH], MSG_EPS, conv_own[0:P, b, :],
                        ALU.add, ALU.add)
                    # h'_b = dent ; transpose -> hpT
                    tp = ptp.tile([128, 128], f32, tag="tp")
                    nc.tensor.transpose(tp[:, 0:P], dent, eye_sb[0:P, 0:P])
                    nc.scalar.copy(hpT[:, b, 0:P], tp[:, 0:P])
                    z = pz.tile([128, 2 * H], f32, tag="z")
                    nc.tensor.matmul(z[0:P, :], hpT[:, b, 0:P],
                                     w1_sb[:, l, :], start=True, stop=False)
                    nc.tensor.matmul(z[0:P, :], ones_sb[0:1, 0:P],
                                     b1_sb[:, l, :], start=False, stop=True)
                    nc.scalar.copy(zs[0:P, b, :], z[0:P, :])
                    ln_block(zs[0:P, b, :], 2 * H, mlpg_sb[0:P, l, :],
                             mlpb_sb[0:P, l, :], sA[0:P, b2H], b,
                             sA[0:P, b2H], sB[0:P, b2H], sA[0:P, b2H])
                    # transposes of u -> uT0, uT1
                    tpa = ptp.tile([128, 128], f32, tag="tp")
                    nc.tensor.transpose(tpa[:, 0:P],
                                        sA[0:P, b * 2 * H:b * 2 * H + H],
                                        eye_sb[0:P, 0:P])
                    nc.scalar.copy(uT0[:, b, 0:P], tpa[:, 0:P])
                    tpb_ = ptp.tile([128, 128], f32, tag="tp")
                    nc.tensor.transpose(tpb_[:, 0:P],
                                        sA[0:P, b * 2 * H + H:(b + 1) * 2 * H],
                                        eye_sb[0:P, 0:P])
                    nc.scalar.copy(uT1[:, b, 0:P], tpb_[:, 0:P])
                    o = po.tile([128, 128], f32, tag="o")
                    nc.tensor.matmul(o[0:P, 0:H], uT0[:, b, 0:P],
                                     w2a_sb[:, l, :], start=True, stop=False)
                    nc.tensor.matmul(o[0:P, 0:H], uT1[:, b, 0:P],
                                     w2b_sb[:, l, :], start=False, stop=False)
                    nc.tensor.matmul(o[0:P, 0:H], ones_sb[0:1, 0:P],
                                     b2_sb[:, l, :], start=False, stop=True)
                    if l == 0:
                        nc.scalar.copy(h_new[0:P, b, :], o[0:P, 0:H])
                    else:
                        nc.vector.tensor_add(h_new[0:P, b, :], o[0:P, 0:H],
                                             h_prev[0:P, b, :])
                    gi = l + 1 if l < L - 1 else 0
                    ln_block(h_new[0:P, b, :], H, lng_sb[0:P, gi, :],
                             lnb_sb[0:P, gi, :], conv_own[0:P, b, :], b,
                             sA[0:P, bH], sD[0:P, bH], sA[0:P, bH])
                    if l < L - 1:
                        hb = 0 if b < 5 else 1
                        if b == 4 or b == NB - 1:
                            nc.gpsimd.dma_start(
                                slab_vs[hb][:, :, :],
                                conv_own[0:P, 5 * hb:5 * hb + 5, :])
                        if b == 4:
                            nc.gpsimd.collective_compute(
                                "AllGather", ALU.bypass,
                                replica_groups=[list(range(NCORES))],
                                ins=[slabs[0].opt()],
                                outs=[tables[l + 1][0][:]],
                            )
                        elif b == NB - 1:
                            nc.gpsimd.collective_compute(
                                "AllGather", ALU.bypass,
                                replica_groups=[list(range(NCORES))],
                                ins=[slabs[1].opt()],
                                outs=[tables[l + 1][1][:]],
                            )

            # ---------------- head ----------------
            pp = po.tile([128, 128], f32, tag="o")
            for b in range(NB):
                nc.tensor.matmul(pp[:, 0:G], conv_own[0:P, b, :],
                                 goh_sb[0:P, b * G:(b + 1) * G],
                                 start=(b == 0), stop=(b == NB - 1))
            nc.scalar.copy(poolT_sb[:], pp[:, 0:G])
            bounce = pdram.tile([128, G], f32)
            nc.sync.dma_start(bounce[:], poolT_sb[:])
            nc.gpsimd.collective_compute(
                "AllReduce", ALU.add,
                replica_groups=[list(range(NCORES))],
                ins=[bounce.opt()], outs=[pool_red[:]],
            )
            nc.sync.dma_start(poolT_sb[:], pool_red[:])
            lg = po.tile([128, 128], f32, tag="o")
            nc.tensor.matmul(lg[0:G, 0:C], poolT_sb[:, 0:G], linW_sb[:],
                             start=True, stop=False)
            nc.tensor.matmul(lg[0:G, 0:C], ones_sb[0:1, 0:G], linb_sb[:],
                             start=False, stop=True)
            nc.scalar.copy(sC[0:G, 0:C], lg[0:G, 0:C])
            nc.sync.dma_start(out_logits[:], sC[0:G, 0:C])
            pl = ptp.tile([128, 128], f32, tag="tp")
            nc.tensor.transpose(pl[0:G, 0:H], poolT_sb[:, 0:G], eye_sb[:])
            nc.scalar.copy(sD[0:G, 0:H], pl[0:G, 0:H])
            nc.sync.dma_start(out_pooled[:], sD[0:G, 0:H])

    # All ACT funcs used here (Relu/Exp/Ln/Copy) live in the
    # natural_log_exp_and_others table set. The load-insertion pass picks the
    # first set containing each func, which alternates exp_and_others /
    # natural_log and thrashes ~2.7us table loads per block. Restrict
    # candidates to the covering set (keeping act_func_set_id positions).
    import concourse.bacc as _bacc_mod
    _orig_tables = _bacc_mod.get_activation_tables

    def _only_nle(arch):
        tabs = _orig_tables(arch)
        return {k: (v if k == "natural_log_exp_and_others" else set())
                for k, v in tabs.items()}

    _bacc_mod.get_activation_tables = _only_nle
    try:
        nc.compile()
    finally:
        _bacc_mod.get_activation_tables = _orig_tables
    return nc


# ----------------------------------------------------------------------------
# host side
# ----------------------------------------------------------------------------

def kernel(**inputs):
    global last_results
    from concourse.bass_utils import run_bass_kernel_spmd

    import ml_dtypes
    bf16_np = ml_dtypes.bfloat16
    f32 = np.float32
    x = np.ascontiguousarray(np.asarray(inputs["x"]), dtype=f32)
    edge_attr = np.ascontiguousarray(np.asarray(inputs["edge_attr"]), dtype=f32)
    ei = np.asarray(inputs["edge_index"]).astype(np.int64)
    batch = np.asarray(inputs["batch"]).astype(np.int64)
    t = np.asarray(inputs["t"], dtype=f32)
    enc_W = np.asarray(inputs["enc_W"], dtype=f32)
    enc_b = np.asarray(inputs["enc_b"], dtype=f32)
    W1 = np.asarray(inputs["W1"], dtype=f32)
    b1 = np.asarray(inputs["b1"], dtype=f32)
    mlp_ln_g = np.asarray(inputs["mlp_ln_g"], dtype=f32)
    mlp_ln_b = np.asarray(inputs["mlp_ln_b"], dtype=f32)
    W2 = np.asarray(inputs["W2"], dtype=f32)
    b2 = np.asarray(inputs["b2"], dtype=f32)
    ln_g = np.asarray(inputs["ln_g"], dtype=f32)
    ln_b = np.asarray(inputs["ln_b"], dtype=f32)
    lin_W = np.asarray(inputs["lin_W"], dtype=f32)
    lin_b = np.asarray(inputs["lin_b"], dtype=f32)

    assert x.shape == (N, H) and edge_attr.shape == (E, H)
    assert ei.shape == (2, E) and batch.shape == (N,)

    src, dst = ei[0], ei[1]
    # src half id: 0 if src is in the first 5 blocks of its owner core
    src_half = ((src % NPC) // (NPC // 2)).astype(np.int64)
    # row of src within its half-table: core*625 + (src_local % 625)
    src_row = (src // NPC) * (NPC // 2) + (src % NPC) % (NPC // 2)
    order = np.lexsort((dst, src_half, dst // BN))
    dsts = dst[order]
    halves = src_half[order]
    rows = src_row[order]
    # per (global block gb, group) counts
    key = (dsts // BN) * 2 + halves
    cntk = np.bincount(key, minlength=2 * (N // BN))
    HTPB = int(np.ceil(cntk.max() / 128))
    HTPB = max(HTPB, 1)
    TPB = 2 * HTPB
    NT = NB * TPB
    E_PAD = NT * 128
    kb = np.searchsorted(key, np.arange(2 * (N // BN) + 1))

    # shared (identical on every core) arrays
    shared = {
        "nvals": np.broadcast_to(np.arange(128, dtype=f32), (128, 128)).astype(bf16_np),
        "eye": np.eye(128, dtype=f32),
        "encW": enc_W.copy(),
        "encb": enc_b.reshape(1, H).copy(),
        "w1": np.ascontiguousarray(W1.transpose(1, 0, 2)),
        "b1": np.ascontiguousarray(b1.reshape(1, L, 2 * H)),
        "w2a": np.ascontiguousarray(W2[:, 0:H, :].transpose(1, 0, 2)),
        "w2b": np.ascontiguousarray(W2[:, H:2 * H, :].transpose(1, 0, 2)),
        "b2": np.ascontiguousarray(b2.reshape(1, L, H)),
        "mlpg": np.ascontiguousarray(
            np.broadcast_to(mlp_ln_g[None], (128, L, 2 * H))),
        "mlpb": np.ascontiguousarray(
            np.broadcast_to(mlp_ln_b[None], (128, L, 2 * H))),
        "lng": np.ascontiguousarray(np.broadcast_to(ln_g[None], (128, L, H))),
        "lnb": np.ascontiguousarray(np.broadcast_to(ln_b[None], (128, L, H))),
        "linW": lin_W.copy(),
        "linb": lin_b.reshape(1, C).copy(),
    }

    xT = np.ascontiguousarray(x.T)
    in_maps = []
    for c in range(NCORES):
        idxg = np.zeros(E_PAD, np.int16)
        drel = np.full(E_PAD, -1.0, f32)
        eap = np.zeros((E_PAD, H), f32)
        for b in range(NB):
            gb = c * NB + b
            for grp in range(2):
                s0 = int(kb[gb * 2 + grp])
                s1 = int(kb[gb * 2 + grp + 1])
                k = s1 - s0
                o0 = (b * TPB + grp * HTPB) * 128
                idxg[o0:o0 + k] = rows[s0:s1].astype(np.int16)
                drel[o0:o0 + k] = (dsts[s0:s1] - gb * BN).astype(f32)
                eap[o0:o0 + k] = edge_attr[order[s0:s1]]
        ea_sh = np.ascontiguousarray(
            eap.reshape(NT, 128, H).transpose(1, 0, 2)).astype(bf16_np)
        drel_sh = np.ascontiguousarray(drel.reshape(NT, 128).T)
        idx_rep = np.ascontiguousarray(
            np.tile(idxg.reshape(E_PAD // 16, 16).T, (8, 1)))
        goh = np.zeros((128, NB * G), f32)
        nloc = np.arange(NPC)
        goh[nloc % BN, (nloc // BN) * G + batch[c * NPC + nloc]] = 1.0
        m = {
            "ea": ea_sh, "idxs": idx_rep, "dstrel": drel_sh,
            "xT": np.ascontiguousarray(xT[:, c * NPC:(c + 1) * NPC]),
            "goh": goh,
        }
        m.update(shared)
        in_maps.append(m)

    key = (TPB, tuple(np.round(t.astype(np.float64), 9).tolist()))
    if key not in _cache:
        _cache[key] = _build_program(TPB, t)
    nc = _cache[key]

    trace = os.environ.get("GNN_TRACE", "0") == "1"
    res = run_bass_kernel_spmd(nc, in_maps, core_ids=list(range(NCORES)),
                               trace=trace)
    last_results = res
    r0 = res.results[0]
    return (np.asarray(r0["out_logits"]), np.asarray(r0["out_pooled"]))
